# revision 29
# baseline (speedup 1.0000x reference)
"""Trainium2 Bass kernel for GRU seq2seq w/ Bahdanau attention (nn_DSkBart).

Sharding: 8-way data-parallel over batch (8 batch cols / core).
Each core runs encoder+decoder recurrence in transposed-state layout
(features on partitions, batch on free) and the big fc_out matmul with
fc_W^T streamed from HBM in bf16, interleaved with the decoder.

Self-contained: hardcodes all shapes; host does layout prep only.
"""

import numpy as np
import ml_dtypes

import concourse.bass as bass
import concourse.bacc as bacc_mod
import concourse.tile as tile
import concourse.mybir as mybir
from concourse.bass_utils import run_bass_kernel_spmd

# problem dims
V, S, T, B = 32000, 64, 64, 64
E, EH, DH = 128, 256, 256
NCORES = 8
BL = B // NCORES          # 8 batch cols per core
TD = T - 1                # 63 decoder steps
M = TD * BL               # 504 output rows per core
MT = 126                  # fc m-tile (4 tiles of 126 = 504)
KX = DH + 2 * EH + E      # 896 = xcat dim
KT_X = KX // 128          # 7
N_CHUNK = 2048            # (unused in vocab-sharded fc)
VS = V // NCORES          # 4000 vocab cols per core
NSUB = 500                # fc psum n-subtile

F32 = mybir.dt.float32
BF16 = mybir.dt.bfloat16
I32 = mybir.dt.int32
AF = mybir.ActivationFunctionType
OP = mybir.AluOpType
bfnp = ml_dtypes.bfloat16

_CACHE = {}


def _build_program(use_cc=True):
    """Build the per-core Bass program (same NEFF on all 8 cores)."""
    nc = bacc_mod.Bacc("TRN2", num_devices=NCORES)

    # ---- DRAM I/O ----
    tok_idx_d = nc.dram_tensor("tok_idx", [128, 8], I32, kind="ExternalInput")
    enc_emb_d = nc.dram_tensor("enc_emb", [V, E], BF16, kind="ExternalInput")
    dec_emb_d = nc.dram_tensor("dec_emb", [V, E], BF16, kind="ExternalInput")
    wihf_d = nc.dram_tensor("wihf_t", [E, 3 * EH], BF16, kind="ExternalInput")
    wihb_d = nc.dram_tensor("wihb_t", [E, 3 * EH], BF16, kind="ExternalInput")
    whhf_d = nc.dram_tensor("whhf_t", [EH, 3 * EH], BF16, kind="ExternalInput")
    whhb_d = nc.dram_tensor("whhb_t", [EH, 3 * EH], BF16, kind="ExternalInput")
    encfc_d = nc.dram_tensor("encfc_t", [2 * EH, DH], BF16, kind="ExternalInput")
    wh_d = nc.dram_tensor("wh_t", [DH, DH], BF16, kind="ExternalInput")
    we_d = nc.dram_tensor("we_t", [2 * EH, DH], BF16, kind="ExternalInput")
    v_d = nc.dram_tensor("v_att", [128, 2], BF16, kind="ExternalInput")
    wihe_d = nc.dram_tensor("wihe_t", [E, 3 * DH], BF16, kind="ExternalInput")
    wihw_d = nc.dram_tensor("wihw_t", [2 * EH, 3 * DH], BF16, kind="ExternalInput")
    whhd_d = nc.dram_tensor("whhd_t", [DH, 3 * DH], BF16, kind="ExternalInput")
    fcwt_d = nc.dram_tensor("fcw_t", [KX, VS], BF16, kind="ExternalInput")
    xg_in = [nc.dram_tensor(f"xg_in{g}", [128, 7, 128], BF16) for g in range(4)]
    xg_out = [nc.dram_tensor(f"xg_out{g}", [NCORES, 128, 7, 128], BF16,
                             addr_space="Shared") for g in range(4)]
    ident_d = nc.dram_tensor("ident", [128, 128], BF16, kind="ExternalInput")
    blk_d = nc.dram_tensor("blkones", [128, 128], F32, kind="ExternalInput")
    out_d = nc.dram_tensor("out", [TD * B, VS], F32, kind="ExternalOutput")

    with tile.TileContext(nc) as tc, \
            tc.tile_pool(name="singles", bufs=1) as sing, \
            tc.tile_pool(name="steps", bufs=2) as stp, \
            tc.tile_pool(name="fcout", bufs=4) as fco_pool, \
            tc.tile_pool(name="pstep", bufs=1, space="PSUM") as pstep:

        def kload(dram, ksub, mdim):
            """Load [K, M] dram weight into sbuf [128, ksub, M]."""
            t = sing.tile([128, ksub, mdim], BF16, tag=dram.name)
            nc.sync.dma_start(t[:], dram[:].rearrange("(ko p) m -> p ko m", p=128))
            return t

        wihf = kload(wihf_d, 1, 768)
        wihb = kload(wihb_d, 1, 768)
        whhf = kload(whhf_d, 2, 768)
        whhb = kload(whhb_d, 2, 768)
        encfc = kload(encfc_d, 4, 256)
        wh = kload(wh_d, 2, 256)
        we = kload(we_d, 4, 256)
        wihe = kload(wihe_d, 1, 768)
        wihw = kload(wihw_d, 4, 768)
        whhd = kload(whhd_d, 2, 768)
        v_sb = sing.tile([128, 2], BF16, tag="v_sb")
        nc.sync.dma_start(v_sb[:], v_d[:])
        ident = sing.tile([128, 128], BF16, tag="ident")
        nc.sync.dma_start(ident[:], ident_d[:])
        blk = sing.tile([128, 128], F32, tag="blk")
        nc.sync.dma_start(blk[:], blk_d[:])

        # persistent activations
        embT_enc = sing.tile([128, 4, 128], BF16, tag="embT_enc")   # [E,(s,b)]
        embT_dec = sing.tile([128, 4, 128], BF16, tag="embT_dec")   # [E,(t,b)]
        encT = sing.tile([128, 4, 512], BF16, tag="encT")           # [2EH,(b,s)] col=b*64+s
        enc_pack = sing.tile([128, 4, 512], BF16, tag="enc_pack")   # [(b%2,s),(b//2,e)]
        enc_proj = sing.tile([128, 2, 512], BF16, tag="enc_proj")   # [DH,(b,s)]
        xcatT = sing.tile([128, 7, 512], BF16, tag="xcatT")         # [896,(t,b)] col=t*8+b
        h_fb_bf = sing.tile([128, 2, 2 * BL], BF16, tag="h_fb_bf")  # enc fwd|bwd state
        h0_bf = sing.tile([128, 2, BL], BF16, tag="h0_bf")
        marker = sing.tile([128, 1], F32, tag="marker")

        nc.vector.memset(h_fb_bf[:], 0.0)
        nc.vector.memset(xcatT[:, :, 504:512], 0.0)

        # ---------- setup: embedding gathers + transposes ----------
        with tc.tile_pool(name="setup", bufs=2) as setp, \
                tc.tile_pool(name="psetup", bufs=1, space="PSUM") as psetp:
            idx_all = sing.tile([128, 8], I32, tag="idx_all")
            nc.sync.dma_start(idx_all[:], tok_idx_d[:])
            for ti, (table, dstT) in enumerate(((enc_emb_d, embT_enc),
                                                (dec_emb_d, embT_dec))):
                for g in range(4):
                    emb_g = setp.tile([128, 128], BF16, tag=f"embg{ti}{g}")
                    nc.gpsimd.indirect_dma_start(
                        out=emb_g[:], out_offset=None, in_=table[:],
                        in_offset=bass.IndirectOffsetOnAxis(
                            ap=idx_all[:, ti * 4 + g:ti * 4 + g + 1], axis=0))
                    pt = psetp.tile([128, 128], BF16, tag="ptrans")
                    nc.tensor.transpose(pt[:], emb_g[:], ident[:])
                    nc.vector.tensor_copy(out=dstT[:, g, :], in_=pt[:])

            # dec embeddings into xcat rows 768:896 (subtile 6)
            for g in range(4):
                nc.vector.tensor_copy(out=xcatT[:, 6, g * 128:(g + 1) * 128],
                                      in_=embT_dec[:, g, :])

        nc.gpsimd.tensor_copy(out=marker[:, 0:1], in_=xcatT[:, 6, 0:1])  # M1

        # ---------- encoder: fwd+bwd GRU, packed gate math ----------
        # sigmoid(x) == (tanh(x/2)+1)/2; gate preacts accumulated on PE:
        #   pg rows 0:4 = (Whh@h + Wih@emb)_rz ; 4:6 = (Wih@emb)_n ; 6:8 = (Whh@h)_n
        def emb_col(embT, s):
            return embT[:, s // 16, (s % 16) * BL:(s % 16 + 1) * BL]

        for i in range(S):
            pg = pstep.tile([128, 8, 2 * BL], F32, tag="gA0")
            for half, whh_t, wih_t, s_pos in ((0, whhf, wihf, i),
                                              (1, whhb, wihb, S - 1 - i)):
                cols = slice(half * BL, (half + 1) * BL)
                hcolv = h_fb_bf[:, :, cols]
                ecol = emb_col(embT_enc, s_pos)
                for mt in range(4):      # rz rows: Whh@h then Wih@emb
                    for kt in range(2):
                        nc.tensor.matmul(pg[:, mt, cols],
                                         lhsT=whh_t[:, kt, mt * 128:(mt + 1) * 128],
                                         rhs=hcolv[:, kt, :],
                                         start=(kt == 0), stop=False,
                                         skip_group_check=True)
                    nc.tensor.matmul(pg[:, mt, cols],
                                     lhsT=wih_t[:, 0, mt * 128:(mt + 1) * 128],
                                     rhs=ecol, start=False, stop=True,
                                     skip_group_check=True)
                for j, mt in enumerate((4, 5)):  # i_n rows: Wih@emb only
                    nc.tensor.matmul(pg[:, 4 + j, cols],
                                     lhsT=wih_t[:, 0, mt * 128:(mt + 1) * 128],
                                     rhs=ecol, start=True, stop=True,
                                     skip_group_check=True)
                for j, mt in enumerate((4, 5)):  # gh_n rows: Whh@h only
                    for kt in range(2):
                        nc.tensor.matmul(pg[:, 6 + j, cols],
                                         lhsT=whh_t[:, kt, mt * 128:(mt + 1) * 128],
                                         rhs=hcolv[:, kt, :],
                                         start=(kt == 0), stop=(kt == 1),
                                         skip_group_check=True)
            th = stp.tile([128, 4, 2 * BL], F32, tag="e_th")
            nc.scalar.activation(out=th[:], in_=pg[:, 0:4, :], func=AF.Tanh, scale=0.5)
            t_n = stp.tile([128, 2, 2 * BL], F32, tag="e_n")
            nc.vector.scalar_tensor_tensor(out=t_n[:], in0=th[:, 0:2, :], scalar=1.0,
                                           in1=pg[:, 6:8, :], op0=OP.add, op1=OP.mult)
            nc.vector.scalar_tensor_tensor(out=t_n[:], in0=t_n[:], scalar=0.5,
                                           in1=pg[:, 4:6, :], op0=OP.mult, op1=OP.add)
            n_t = stp.tile([128, 2, 2 * BL], F32, tag="e_tanh")
            nc.scalar.activation(out=n_t[:], in_=t_n[:], func=AF.Tanh)
            d_t = stp.tile([128, 2, 2 * BL], F32, tag="e_d")
            nc.vector.tensor_tensor(out=d_t[:], in0=h_fb_bf[:], in1=n_t[:],
                                    op=OP.subtract)
            nc.vector.scalar_tensor_tensor(out=d_t[:], in0=th[:, 2:4, :], scalar=1.0,
                                           in1=d_t[:], op0=OP.add, op1=OP.mult)
            nc.vector.scalar_tensor_tensor(out=h_fb_bf[:], in0=d_t[:], scalar=0.5,
                                           in1=n_t[:], op0=OP.mult, op1=OP.add)
            nc.vector.tensor_copy(out=encT[:, 0:2, i::64], in_=h_fb_bf[:, :, 0:BL])
            nc.vector.tensor_copy(out=encT[:, 2:4, (S - 1 - i)::64],
                                  in_=h_fb_bf[:, :, BL:2 * BL])

        # hidden0 = tanh(encfc_W @ [hf; hb])
        hcat = stp.tile([128, 4, BL], BF16, tag="hcat")
        nc.vector.tensor_copy(out=hcat[:, 0:2, :], in_=h_fb_bf[:, :, 0:BL])
        nc.vector.tensor_copy(out=hcat[:, 2:4, :], in_=h_fb_bf[:, :, BL:2 * BL])
        ph0 = pstep.tile([128, 2, BL], F32, tag="g10")
        for mt in range(2):
            for kt in range(4):
                nc.tensor.matmul(ph0[:, mt, :],
                                 lhsT=encfc[:, kt, mt * 128:(mt + 1) * 128],
                                 rhs=hcat[:, kt, :], start=(kt == 0), stop=(kt == 3))
        nc.scalar.activation(out=h0_bf[:], in_=ph0[:], func=AF.Tanh)
        nc.gpsimd.tensor_copy(out=marker[:, 0:1], in_=h0_bf[:, 0, 0:1])  # M2

        # ---------- attention precompute ----------
        with tc.tile_pool(name="prep2", bufs=1, space="PSUM") as pp2:
            # enc_proj^T [DH,(b,s)] = We^T.T @ encT
            for mt in range(2):
                pe = pp2.tile([128, 512], F32, tag="pproj")
                for kt in range(4):
                    nc.tensor.matmul(pe[:], lhsT=we[:, kt, mt * 128:(mt + 1) * 128],
                                     rhs=encT[:, kt, :], start=(kt == 0), stop=(kt == 3))
                nc.vector.tensor_copy(out=enc_proj[:, mt, :], in_=pe[:])
            # enc_pack [(b%2)*64+s, (b//2, e)] via 16 PE transposes of [128,128]
            for et in range(4):
                for bp in range(4):
                    ptp = pp2.tile([128, 128], BF16, tag="ppack")
                    nc.tensor.transpose(ptp[:], encT[:, et, bp * 128:(bp + 1) * 128],
                                        ident[:])
                    nc.vector.tensor_copy(
                        out=enc_pack[:, bp, et * 128:(et + 1) * 128], in_=ptp[:])

        # ---------- decoder: two half-batch chains + vocab-sharded fc ----------
        # fc_out: each core holds fc_W^T[:, shard] resident; xcat gathered
        # from all cores per 16-step m-group via AllGather.
        with tc.tile_pool(name="fcps", bufs=2, space="PSUM") as fcps, \
                tc.tile_pool(name="fcg", bufs=2) as fcg_pool:
            fcw_sb = sing.tile([128, 7, VS], BF16, tag="fcw_sb")
            nc.sync.dma_start(fcw_sb[:],
                              fcwt_d[:].rearrange("(ko p) n -> p ko n", p=128))
            fc_queue = []
            fc_flip = [0]
            xg_tiles = {}
            HB = BL // 2  # 4 batch cols per chain

            def emit_gather(g):
                nc.sync.dma_start(xg_in[g][:],
                                  xcatT[:, :, 16 * g * BL:(16 * g + 16) * BL])
                if use_cc:
                    nc.gpsimd.collective_compute(
                        "AllGather", OP.bypass,
                        replica_groups=[list(range(NCORES))],
                        ins=[xg_in[g].ap()], outs=[xg_out[g].ap()])
                    src_v = xg_out[g][:].rearrange("r p k (t b) -> p k t r b", b=BL)
                else:  # sim-only stand-in: replicate own slice 8x
                    src_v = xg_in[g][:].rearrange("p k (t b) -> p k t b", b=BL)
                xg = fcg_pool.tile([128, 7, 16, NCORES, BL], BF16, tag="xg_sb")
                if use_cc:
                    nc.sync.dma_start(xg[:], src_v)
                else:
                    for r in range(NCORES):
                        nc.sync.dma_start(xg[:, :, :, r, :], src_v)
                xg_tiles[g] = xg[:].rearrange("p k t r b -> p k (t r b)")

            def emit_fc_unit(g, mt, ns):
                xg = xg_tiles[g]
                cs = min(NSUB, VS - ns * NSUB)
                row0 = g * 1024 + mt * 128
                rows = min(128, TD * B - row0)   # last group tile is 64 rows
                ps = fcps.tile([128, NSUB], F32, tag="fcp")
                for kt in range(KT_X):
                    nc.tensor.matmul(
                        ps[:, :cs],
                        lhsT=xg[:, kt, mt * 128:(mt + 1) * 128],
                        rhs=fcw_sb[:, kt, ns * NSUB:ns * NSUB + cs],
                        start=(kt == 0), stop=(kt == KT_X - 1))
                osb = fco_pool.tile([128, NSUB], F32, tag="osb")
                fc_flip[0] ^= 1
                if fc_flip[0]:
                    nc.vector.tensor_copy(out=osb[:rows, :cs], in_=ps[:rows, :cs])
                else:
                    nc.scalar.copy(out=osb[:rows, :cs], in_=ps[:rows, :cs])
                nc.sync.dma_start(
                    out_d[row0:row0 + rows, ns * NSUB:ns * NSUB + cs],
                    osb[:rows, :cs])

            def dec_step(t, c):
                bb = c * HB                       # batch base within the 8
                h_prev = h0_bf[:, :, bb:bb + HB] if t == 0 else \
                    xcatT[:, 0:2, (t - 1) * BL + bb:(t - 1) * BL + bb + HB]
                # q^T [DH, hb]
                pq = pstep.tile([128, 2, HB], F32, tag=f"g1{c}")
                for mt in range(2):
                    for kt in range(2):
                        nc.tensor.matmul(pq[:, mt, :],
                                         lhsT=wh[:, kt, mt * 128:(mt + 1) * 128],
                                         rhs=h_prev[:, kt, :],
                                         start=(kt == 0), stop=(kt == 1))
                q_bf = stp.tile([128, 2, HB], BF16, tag=f"q_bf{c}")
                nc.vector.tensor_copy(out=q_bf[:], in_=pq[:])
                # energy = tanh(enc_proj + q), chain's (b,s) cols
                epv = enc_proj[:, :, bb * 64:(bb + HB) * 64]
                energy = stp.tile([128, 2, HB * 64], BF16, tag=f"energy{c}")
                nc.vector.tensor_tensor(
                    out=energy[:].rearrange("p k (b s) -> p k b s", s=64),
                    in0=epv.rearrange("p k (b s) -> p k b s", s=64),
                    in1=q_bf[:, :, :, None].to_broadcast([128, 2, HB, 64]),
                    op=OP.add)
                nc.scalar.activation(out=energy[:], in_=energy[:], func=AF.Tanh)
                # scores [(b%2)*64+s, local pair j]
                psc = pstep.tile([128, 2], F32, tag=f"g1{c}")
                for j in range(2):
                    for kt in range(2):
                        nc.tensor.matmul(
                            psc[:, j:j + 1],
                            lhsT=energy[:, kt, j * 128:(j + 1) * 128],
                            rhs=v_sb[:, kt:kt + 1], start=(kt == 0), stop=(kt == 1))
                exp_f = stp.tile([128, 2], F32, tag=f"exp_f{c}")
                nc.scalar.activation(out=exp_f[:], in_=psc[:], func=AF.Exp)
                pz = pstep.tile([128, 2], F32, tag=f"g1{c}")
                nc.tensor.matmul(pz[:], lhsT=blk[:], rhs=exp_f[:], start=True,
                                 stop=True)
                rcp = stp.tile([128, 2], F32, tag=f"rcp{c}")
                nc.vector.reciprocal(out=rcp[:], in_=pz[:])
                a_eo = a_eo_c[c]
                nc.vector.tensor_tensor(out=a_eo[0:64, :, 0], in0=exp_f[0:64, :],
                                        in1=rcp[0:64, :], op=OP.mult)
                nc.vector.tensor_tensor(out=a_eo[64:128, :, 1], in0=exp_f[64:128, :],
                                        in1=rcp[64:128, :], op=OP.mult)
                # weighted^T [2EH, hb]
                pw = pstep.tile([128, 4, HB], F32, tag=f"g3{c}")
                for j in range(2):
                    b2 = 2 * c + j
                    for et in range(4):
                        nc.tensor.matmul(
                            pw[:, et, 2 * j:2 * j + 2],
                            lhsT=enc_pack[:, b2, et * 128:(et + 1) * 128],
                            rhs=a_eo[:, j, :], start=True, stop=True)
                wdst = xcatT[:, 2:6, t * BL + bb:t * BL + bb + HB]
                nc.vector.tensor_copy(out=wdst, in_=pw[:])
                # gate preacts on PE: rows 0:4 rz-sum, 4:6 i_n, 6:8 gh_n
                pg = pstep.tile([128, 8, HB], F32, tag=f"gA{c}")
                ecol = embT_dec[:, t // 16, (t % 16) * BL + bb:(t % 16) * BL + bb + HB]
                for mt in range(4):
                    for kt in range(2):
                        nc.tensor.matmul(pg[:, mt, :],
                                         lhsT=whhd[:, kt, mt * 128:(mt + 1) * 128],
                                         rhs=h_prev[:, kt, :],
                                         start=(kt == 0), stop=False,
                                         skip_group_check=True)
                    nc.tensor.matmul(pg[:, mt, :],
                                     lhsT=wihe[:, 0, mt * 128:(mt + 1) * 128],
                                     rhs=ecol, start=False, stop=False,
                                     skip_group_check=True)
                    for kt in range(4):
                        nc.tensor.matmul(pg[:, mt, :],
                                         lhsT=wihw[:, kt, mt * 128:(mt + 1) * 128],
                                         rhs=wdst[:, kt, :],
                                         start=False, stop=(kt == 3),
                                         skip_group_check=True)
                for j2, mt in enumerate((4, 5)):   # i_n = Wihe@emb + Wihw@w
                    nc.tensor.matmul(pg[:, 4 + j2, :],
                                     lhsT=wihe[:, 0, mt * 128:(mt + 1) * 128],
                                     rhs=ecol, start=True, stop=False,
                                     skip_group_check=True)
                    for kt in range(4):
                        nc.tensor.matmul(pg[:, 4 + j2, :],
                                         lhsT=wihw[:, kt, mt * 128:(mt + 1) * 128],
                                         rhs=wdst[:, kt, :],
                                         start=False, stop=(kt == 3),
                                         skip_group_check=True)
                for j2, mt in enumerate((4, 5)):   # gh_n
                    for kt in range(2):
                        nc.tensor.matmul(pg[:, 6 + j2, :],
                                         lhsT=whhd[:, kt, mt * 128:(mt + 1) * 128],
                                         rhs=h_prev[:, kt, :],
                                         start=(kt == 0), stop=(kt == 1),
                                         skip_group_check=True)
                # gates (sigmoid via tanh(x/2))
                th = stp.tile([128, 4, HB], F32, tag=f"d_th{c}")
                nc.scalar.activation(out=th[:], in_=pg[:, 0:4, :], func=AF.Tanh,
                                     scale=0.5)
                t_n = stp.tile([128, 2, HB], F32, tag=f"d_n{c}")
                nc.vector.scalar_tensor_tensor(out=t_n[:], in0=th[:, 0:2, :],
                                               scalar=1.0, in1=pg[:, 6:8, :],
                                               op0=OP.add, op1=OP.mult)
                nc.vector.scalar_tensor_tensor(out=t_n[:], in0=t_n[:], scalar=0.5,
                                               in1=pg[:, 4:6, :], op0=OP.mult,
                                               op1=OP.add)
                n_t = stp.tile([128, 2, HB], F32, tag=f"d_tanh{c}")
                nc.scalar.activation(out=n_t[:], in_=t_n[:], func=AF.Tanh)
                d_t = stp.tile([128, 2, HB], F32, tag=f"d_d{c}")
                nc.vector.tensor_tensor(out=d_t[:], in0=h_prev, in1=n_t[:],
                                        op=OP.subtract)
                nc.vector.scalar_tensor_tensor(out=d_t[:], in0=th[:, 2:4, :],
                                               scalar=1.0, in1=d_t[:], op0=OP.add,
                                               op1=OP.mult)
                nc.vector.scalar_tensor_tensor(
                    out=xcatT[:, 0:2, t * BL + bb:t * BL + bb + HB], in0=d_t[:],
                    scalar=0.5, in1=n_t[:], op0=OP.mult, op1=OP.add)

            a_eo_c = []
            for c in range(2):
                ae = sing.tile([128, 2, 2], BF16, tag=f"a_eoc{c}", name=f"a_eoc{c}")
                nc.vector.memset(ae[:], 0.0)
                a_eo_c.append(ae)

            for t in range(TD):
                dec_step(t, 0)
                dec_step(t, 1)
                if t in (15, 31, 47):
                    g = t // 16
                    emit_gather(g)
                    fc_queue.extend((g, mt, ns)
                                    for mt in range(8) for ns in range(8))
                for _ in range(4):
                    if fc_queue:
                        emit_fc_unit(*fc_queue.pop(0))
            nc.gpsimd.tensor_copy(out=marker[:, 0:1],
                                  in_=xcatT[:, 0, (TD - 1) * BL:(TD - 1) * BL + 1])
            emit_gather(3)
            fc_queue.extend((3, mt, ns) for mt in range(8) for ns in range(8))
            for item in fc_queue:
                emit_fc_unit(*item)

    nc.compile()
    return nc


def _prep_inputs(inputs):
    """Host-side layout prep shared across cores. Returns (shared, per_core)."""
    f = {k: np.asarray(v) for k, v in inputs.items()}
    bf = lambda a: np.ascontiguousarray(a, dtype=np.float32).astype(bfnp)
    tr = lambda a: bf(np.asarray(a, np.float32).T)

    shared = dict(
        enc_emb=bf(f["enc_emb"]),
        dec_emb=bf(f["dec_emb"]),
        wihf_t=tr(f["enc_Wih_f"]), wihb_t=tr(f["enc_Wih_b"]),
        whhf_t=tr(f["enc_Whh_f"]), whhb_t=tr(f["enc_Whh_b"]),
        encfc_t=tr(f["enc_fc_W"]),
        wh_t=tr(f["attn_W"][:, :DH]), we_t=tr(f["attn_W"][:, DH:]),
        v_att=bf(f["attn_v"][0].reshape(2, 128).T),
        wihe_t=tr(f["dec_Wih"][:, :E]), wihw_t=tr(f["dec_Wih"][:, E:]),
        whhd_t=tr(f["dec_Whh"]),
        ident=np.eye(128, dtype=bfnp),
        blkones=np.kron(np.eye(2, dtype=np.float32), np.ones((64, 64), np.float32)),
    )

    src = np.asarray(f["src"])
    trg = np.asarray(f["trg"])
    fcwt_full = tr(f["fc_W"])                                     # [896, 32000]
    per_core = []
    for c in range(NCORES):
        cols = slice(c * BL, (c + 1) * BL)
        si = src[:, cols].astype(np.int32).reshape(-1)            # s-major, 512
        ti = trg[:TD, cols].astype(np.int32).reshape(-1)          # t-major, 504
        ti = np.concatenate([ti, np.zeros(8, np.int32)])
        tok = np.concatenate([si.reshape(4, 128), ti.reshape(4, 128)]).T  # [128, 8]
        per_core.append(dict(
            tok_idx=np.ascontiguousarray(tok),
            fcw_t=np.ascontiguousarray(fcwt_full[:, c * VS:(c + 1) * VS])))
    return shared, per_core


def kernel(**inputs):
    if "nc" not in _CACHE:
        _CACHE["nc"] = _build_program()
    nc = _CACHE["nc"]

    shared, per_core = _prep_inputs(inputs)
    in_maps = [{**shared, **pc} for pc in per_core]

    res = run_bass_kernel_spmd(nc, in_maps, core_ids=list(range(NCORES)))
    _CACHE["last_result"] = res

    out = np.zeros((T, B, V), np.float32)
    for c in range(NCORES):
        out[1:, :, c * VS:(c + 1) * VS] = res.results[c]["out"].reshape(TD, B, VS)
    return out


# revision 33
# speedup vs baseline: 1.0354x; 1.0354x over previous
"""Trainium2 Bass kernel for GRU seq2seq w/ Bahdanau attention (nn_DSkBart).

Sharding: 8-way data-parallel over batch (8 batch cols / core).
Each core runs encoder+decoder recurrence in transposed-state layout
(features on partitions, batch on free) and the big fc_out matmul with
fc_W^T streamed from HBM in bf16, interleaved with the decoder.

Self-contained: hardcodes all shapes; host does layout prep only.
"""

import numpy as np
import ml_dtypes

import concourse.bass as bass
import concourse.bacc as bacc_mod
import concourse.tile as tile
import concourse.mybir as mybir
from concourse.bass_utils import run_bass_kernel_spmd

# problem dims
V, S, T, B = 32000, 64, 64, 64
E, EH, DH = 128, 256, 256
NCORES = 8
BL = B // NCORES          # 8 batch cols per core
TD = T - 1                # 63 decoder steps
M = TD * BL               # 504 output rows per core
MT = 126                  # fc m-tile (4 tiles of 126 = 504)
KX = DH + 2 * EH + E      # 896 = xcat dim
KT_X = KX // 128          # 7
N_CHUNK = 2048            # (unused in vocab-sharded fc)
VS = V // NCORES          # 4000 vocab cols per core
NSUB = 500                # fc psum n-subtile

F32 = mybir.dt.float32
BF16 = mybir.dt.bfloat16
I32 = mybir.dt.int32
AF = mybir.ActivationFunctionType
OP = mybir.AluOpType
bfnp = ml_dtypes.bfloat16

_CACHE = {}


def _build_program(use_cc=True):
    """Build the per-core Bass program (same NEFF on all 8 cores)."""
    nc = bacc_mod.Bacc("TRN2", num_devices=NCORES)

    # ---- DRAM I/O ----
    tok_idx_d = nc.dram_tensor("tok_idx", [128, 8], I32, kind="ExternalInput")
    enc_emb_d = nc.dram_tensor("enc_emb", [V, E], BF16, kind="ExternalInput")
    dec_emb_d = nc.dram_tensor("dec_emb", [V, E], BF16, kind="ExternalInput")
    wihf_d = nc.dram_tensor("wihf_t", [E, 3 * EH], BF16, kind="ExternalInput")
    wihb_d = nc.dram_tensor("wihb_t", [E, 3 * EH], BF16, kind="ExternalInput")
    whhf_d = nc.dram_tensor("whhf_t", [EH, 3 * EH], BF16, kind="ExternalInput")
    whhb_d = nc.dram_tensor("whhb_t", [EH, 3 * EH], BF16, kind="ExternalInput")
    encfc_d = nc.dram_tensor("encfc_t", [2 * EH, DH], BF16, kind="ExternalInput")
    wh_d = nc.dram_tensor("wh_t", [DH, DH], BF16, kind="ExternalInput")
    we_d = nc.dram_tensor("we_t", [2 * EH, DH], BF16, kind="ExternalInput")
    v_d = nc.dram_tensor("v_att", [128, 2], BF16, kind="ExternalInput")
    wihe_d = nc.dram_tensor("wihe_t", [E, 3 * DH], BF16, kind="ExternalInput")
    wihw_d = nc.dram_tensor("wihw_t", [2 * EH, 3 * DH], BF16, kind="ExternalInput")
    whhd_d = nc.dram_tensor("whhd_t", [DH, 3 * DH], BF16, kind="ExternalInput")
    fcwt_d = nc.dram_tensor("fcw_t", [KX, VS], BF16, kind="ExternalInput")
    GATHERS = ((15, 0, 16), (31, 16, 16), (47, 32, 16), (55, 48, 8), (62, 56, 8))
    xg_in = [nc.dram_tensor(f"xg_in{g}", [128, 7, 8 * ns], BF16)
             for g, (_, _, ns) in enumerate(GATHERS)]
    xg_out = [nc.dram_tensor(f"xg_out{g}", [NCORES, 128, 7, 8 * ns], BF16,
                             addr_space="Shared") for g, (_, _, ns) in enumerate(GATHERS)]
    ident_d = nc.dram_tensor("ident", [128, 128], BF16, kind="ExternalInput")
    blk_d = nc.dram_tensor("blkones", [128, 128], F32, kind="ExternalInput")
    out_d = nc.dram_tensor("out", [TD * B, VS], F32, kind="ExternalOutput")

    with tile.TileContext(nc) as tc, \
            tc.tile_pool(name="singles", bufs=1) as sing, \
            tc.tile_pool(name="steps", bufs=2) as stp, \
            tc.tile_pool(name="fcout", bufs=4) as fco_pool, \
            tc.tile_pool(name="pstep", bufs=1, space="PSUM") as pstep:

        def kload(dram, ksub, mdim):
            """Load [K, M] dram weight into sbuf [128, ksub, M]."""
            t = sing.tile([128, ksub, mdim], BF16, tag=dram.name)
            nc.sync.dma_start(t[:], dram[:].rearrange("(ko p) m -> p ko m", p=128))
            return t

        wihf = kload(wihf_d, 1, 768)
        wihb = kload(wihb_d, 1, 768)
        whhf = kload(whhf_d, 2, 768)
        whhb = kload(whhb_d, 2, 768)
        encfc = kload(encfc_d, 4, 256)
        wh = kload(wh_d, 2, 256)
        we = kload(we_d, 4, 256)
        wihe = kload(wihe_d, 1, 768)
        wihw = kload(wihw_d, 4, 768)
        whhd = kload(whhd_d, 2, 768)
        v_sb = sing.tile([128, 2], BF16, tag="v_sb")
        nc.sync.dma_start(v_sb[:], v_d[:])
        ident = sing.tile([128, 128], BF16, tag="ident")
        nc.sync.dma_start(ident[:], ident_d[:])
        blk = sing.tile([128, 128], F32, tag="blk")
        nc.sync.dma_start(blk[:], blk_d[:])

        # persistent activations
        embT_enc = sing.tile([128, 4, 128], BF16, tag="embT_enc")   # [E,(s,b)]
        embT_dec = sing.tile([128, 4, 128], BF16, tag="embT_dec")   # [E,(t,b)]
        encT = sing.tile([128, 4, 512], BF16, tag="encT")           # [2EH,(b,s)] col=b*64+s
        enc_pack = sing.tile([128, 4, 512], BF16, tag="enc_pack")   # [(b%2,s),(b//2,e)]
        enc_proj = sing.tile([128, 2, 512], BF16, tag="enc_proj")   # [DH,(b,s)]
        xcatT = sing.tile([128, 7, 512], BF16, tag="xcatT")         # [896,(t,b)] col=t*8+b
        h_fb_bf = sing.tile([128, 2, 2 * BL], BF16, tag="h_fb_bf")  # enc fwd|bwd state
        h0_bf = sing.tile([128, 2, BL], BF16, tag="h0_bf")
        marker = sing.tile([128, 1], F32, tag="marker")

        nc.vector.memset(h_fb_bf[:], 0.0)
        nc.vector.memset(xcatT[:, :, 504:512], 0.0)

        # ---------- setup: embedding gathers + transposes ----------
        with tc.tile_pool(name="setup", bufs=2) as setp, \
                tc.tile_pool(name="psetup", bufs=1, space="PSUM") as psetp:
            idx_all = sing.tile([128, 8], I32, tag="idx_all")
            nc.sync.dma_start(idx_all[:], tok_idx_d[:])
            for ti, (table, dstT) in enumerate(((enc_emb_d, embT_enc),
                                                (dec_emb_d, embT_dec))):
                for g in range(4):
                    emb_g = setp.tile([128, 128], BF16, tag=f"embg{ti}{g}")
                    nc.gpsimd.indirect_dma_start(
                        out=emb_g[:], out_offset=None, in_=table[:],
                        in_offset=bass.IndirectOffsetOnAxis(
                            ap=idx_all[:, ti * 4 + g:ti * 4 + g + 1], axis=0))
                    pt = psetp.tile([128, 128], BF16, tag="ptrans")
                    nc.tensor.transpose(pt[:], emb_g[:], ident[:])
                    nc.vector.tensor_copy(out=dstT[:, g, :], in_=pt[:])

            # dec embeddings into xcat rows 768:896 (subtile 6)
            for g in range(4):
                nc.vector.tensor_copy(out=xcatT[:, 6, g * 128:(g + 1) * 128],
                                      in_=embT_dec[:, g, :])

        nc.gpsimd.tensor_copy(out=marker[:, 0:1], in_=xcatT[:, 6, 0:1])  # M1

        # ---------- encoder: fwd+bwd GRU, packed gate math ----------
        # sigmoid(x) == (tanh(x/2)+1)/2; gate preacts accumulated on PE:
        #   pg rows 0:4 = (Whh@h + Wih@emb)_rz ; 4:6 = (Wih@emb)_n ; 6:8 = (Whh@h)_n
        def emb_col(embT, s):
            return embT[:, s // 16, (s % 16) * BL:(s % 16 + 1) * BL]

        for i in range(S):
            pg = pstep.tile([128, 8, 2 * BL], F32, tag="gA0")
            for half, whh_t, wih_t, s_pos in ((0, whhf, wihf, i),
                                              (1, whhb, wihb, S - 1 - i)):
                cols = slice(half * BL, (half + 1) * BL)
                hcolv = h_fb_bf[:, :, cols]
                ecol = emb_col(embT_enc, s_pos)
                for mt in range(4):      # rz rows: Whh@h then Wih@emb
                    for kt in range(2):
                        nc.tensor.matmul(pg[:, mt, cols],
                                         lhsT=whh_t[:, kt, mt * 128:(mt + 1) * 128],
                                         rhs=hcolv[:, kt, :],
                                         start=(kt == 0), stop=False,
                                         skip_group_check=True)
                    nc.tensor.matmul(pg[:, mt, cols],
                                     lhsT=wih_t[:, 0, mt * 128:(mt + 1) * 128],
                                     rhs=ecol, start=False, stop=True,
                                     skip_group_check=True)
                for j, mt in enumerate((4, 5)):  # i_n rows: Wih@emb only
                    nc.tensor.matmul(pg[:, 4 + j, cols],
                                     lhsT=wih_t[:, 0, mt * 128:(mt + 1) * 128],
                                     rhs=ecol, start=True, stop=True,
                                     skip_group_check=True)
                for j, mt in enumerate((4, 5)):  # gh_n rows: Whh@h only
                    for kt in range(2):
                        nc.tensor.matmul(pg[:, 6 + j, cols],
                                         lhsT=whh_t[:, kt, mt * 128:(mt + 1) * 128],
                                         rhs=hcolv[:, kt, :],
                                         start=(kt == 0), stop=(kt == 1),
                                         skip_group_check=True)
            th = stp.tile([128, 4, 2 * BL], F32, tag="e_th")
            nc.scalar.activation(out=th[:], in_=pg[:, 0:4, :], func=AF.Tanh, scale=0.5)
            t_n = stp.tile([128, 2, 2 * BL], F32, tag="e_n")
            nc.vector.scalar_tensor_tensor(out=t_n[:], in0=th[:, 0:2, :], scalar=1.0,
                                           in1=pg[:, 6:8, :], op0=OP.add, op1=OP.mult)
            nc.vector.scalar_tensor_tensor(out=t_n[:], in0=t_n[:], scalar=0.5,
                                           in1=pg[:, 4:6, :], op0=OP.mult, op1=OP.add)
            n_t = stp.tile([128, 2, 2 * BL], F32, tag="e_tanh")
            nc.scalar.activation(out=n_t[:], in_=t_n[:], func=AF.Tanh)
            d_t = stp.tile([128, 2, 2 * BL], F32, tag="e_d")
            nc.vector.tensor_tensor(out=d_t[:], in0=h_fb_bf[:], in1=n_t[:],
                                    op=OP.subtract)
            nc.vector.scalar_tensor_tensor(out=d_t[:], in0=th[:, 2:4, :], scalar=1.0,
                                           in1=d_t[:], op0=OP.add, op1=OP.mult)
            nc.vector.scalar_tensor_tensor(out=h_fb_bf[:], in0=d_t[:], scalar=0.5,
                                           in1=n_t[:], op0=OP.mult, op1=OP.add)
            nc.vector.tensor_copy(out=encT[:, 0:2, i::64], in_=h_fb_bf[:, :, 0:BL])
            nc.vector.tensor_copy(out=encT[:, 2:4, (S - 1 - i)::64],
                                  in_=h_fb_bf[:, :, BL:2 * BL])

        # hidden0 = tanh(encfc_W @ [hf; hb])
        hcat = stp.tile([128, 4, BL], BF16, tag="hcat")
        nc.vector.tensor_copy(out=hcat[:, 0:2, :], in_=h_fb_bf[:, :, 0:BL])
        nc.vector.tensor_copy(out=hcat[:, 2:4, :], in_=h_fb_bf[:, :, BL:2 * BL])
        ph0 = pstep.tile([128, 2, BL], F32, tag="g10")
        for mt in range(2):
            for kt in range(4):
                nc.tensor.matmul(ph0[:, mt, :],
                                 lhsT=encfc[:, kt, mt * 128:(mt + 1) * 128],
                                 rhs=hcat[:, kt, :], start=(kt == 0), stop=(kt == 3))
        nc.scalar.activation(out=h0_bf[:], in_=ph0[:], func=AF.Tanh)
        nc.gpsimd.tensor_copy(out=marker[:, 0:1], in_=h0_bf[:, 0, 0:1])  # M2

        # ---------- attention precompute ----------
        with tc.tile_pool(name="prep2", bufs=1, space="PSUM") as pp2:
            # enc_proj^T [DH,(b,s)] = We^T.T @ encT
            for mt in range(2):
                pe = pp2.tile([128, 512], F32, tag="pproj")
                for kt in range(4):
                    nc.tensor.matmul(pe[:], lhsT=we[:, kt, mt * 128:(mt + 1) * 128],
                                     rhs=encT[:, kt, :], start=(kt == 0), stop=(kt == 3))
                nc.vector.tensor_copy(out=enc_proj[:, mt, :], in_=pe[:])
            # enc_pack [(b%2)*64+s, (b//2, e)] via 16 PE transposes of [128,128]
            for et in range(4):
                for bp in range(4):
                    ptp = pp2.tile([128, 128], BF16, tag="ppack")
                    nc.tensor.transpose(ptp[:], encT[:, et, bp * 128:(bp + 1) * 128],
                                        ident[:])
                    nc.vector.tensor_copy(
                        out=enc_pack[:, bp, et * 128:(et + 1) * 128], in_=ptp[:])

        # ---------- decoder: two half-batch chains + vocab-sharded fc ----------
        # fc_out: each core holds fc_W^T[:, shard] resident; xcat gathered
        # from all cores per 16-step m-group via AllGather.
        with tc.tile_pool(name="fcps", bufs=2, space="PSUM") as fcps, \
                tc.tile_pool(name="fcg", bufs=2) as fcg_pool:
            fcw_sb = sing.tile([128, 7, VS], BF16, tag="fcw_sb")
            nc.sync.dma_start(fcw_sb[:],
                              fcwt_d[:].rearrange("(ko p) n -> p ko n", p=128))
            fc_queue = []
            fc_flip = [0]
            xg_tiles = {}
            HB = BL // 2  # 4 batch cols per chain

            def emit_gather(g):
                _, t0s, nst = GATHERS[g]
                nc.sync.dma_start(xg_in[g][:],
                                  xcatT[:, :, t0s * BL:(t0s + nst) * BL])
                if use_cc:
                    nc.gpsimd.collective_compute(
                        "AllGather", OP.bypass,
                        replica_groups=[list(range(NCORES))],
                        ins=[xg_in[g].ap()], outs=[xg_out[g].ap()])
                    src_v = xg_out[g][:].rearrange("r p k (t b) -> p k t r b", b=BL)
                else:  # sim-only stand-in: replicate own slice 8x
                    src_v = xg_in[g][:].rearrange("p k (t b) -> p k t b", b=BL)
                xg = fcg_pool.tile([128, 7, nst, NCORES, BL], BF16,
                                   tag=f"xg_sb{nst}", name=f"xg{nst}")
                if use_cc:
                    nc.sync.dma_start(xg[:], src_v)
                else:
                    nc.sync.dma_start(
                        xg[:].rearrange("p k t r b -> p k (t r b)")[:, :, :8 * nst],
                        xg_in[g][:])
                xg_tiles[g] = xg[:].rearrange("p k t r b -> p k (t r b)")

            def emit_fc_unit(g, mt, ns):
                xg = xg_tiles[g]
                cs = min(NSUB, VS - ns * NSUB)
                row0 = GATHERS[g][1] * B + mt * 128
                rows = min(128, TD * B - row0)   # last group tile is 64 rows
                ps = fcps.tile([128, NSUB], F32, tag="fcp")
                for kt in range(KT_X):
                    nc.tensor.matmul(
                        ps[:, :cs],
                        lhsT=xg[:, kt, mt * 128:(mt + 1) * 128],
                        rhs=fcw_sb[:, kt, ns * NSUB:ns * NSUB + cs],
                        start=(kt == 0), stop=(kt == KT_X - 1))
                osb = fco_pool.tile([128, NSUB], F32, tag="osb")
                fc_flip[0] ^= 1
                if fc_flip[0]:
                    nc.vector.tensor_copy(out=osb[:rows, :cs], in_=ps[:rows, :cs])
                else:
                    nc.scalar.copy(out=osb[:rows, :cs], in_=ps[:rows, :cs])
                nc.sync.dma_start(
                    out_d[row0:row0 + rows, ns * NSUB:ns * NSUB + cs],
                    osb[:rows, :cs])

            def dec_step(t, c):
                bb = c * HB                       # batch base within the 8
                h_prev = h0_bf[:, :, bb:bb + HB] if t == 0 else \
                    xcatT[:, 0:2, (t - 1) * BL + bb:(t - 1) * BL + bb + HB]
                # q^T [DH, hb]
                pq = pstep.tile([128, 2, HB], F32, tag=f"g1{c}")
                for mt in range(2):
                    for kt in range(2):
                        nc.tensor.matmul(pq[:, mt, :],
                                         lhsT=wh[:, kt, mt * 128:(mt + 1) * 128],
                                         rhs=h_prev[:, kt, :],
                                         start=(kt == 0), stop=(kt == 1))
                q_bf = stp.tile([128, 2, HB], BF16, tag=f"q_bf{c}")
                nc.vector.tensor_copy(out=q_bf[:], in_=pq[:])
                # energy = tanh(enc_proj + q), chain's (b,s) cols
                epv = enc_proj[:, :, bb * 64:(bb + HB) * 64]
                energy = stp.tile([128, 2, HB * 64], BF16, tag=f"energy{c}")
                nc.vector.tensor_tensor(
                    out=energy[:].rearrange("p k (b s) -> p k b s", s=64),
                    in0=epv.rearrange("p k (b s) -> p k b s", s=64),
                    in1=q_bf[:, :, :, None].to_broadcast([128, 2, HB, 64]),
                    op=OP.add)
                nc.scalar.activation(out=energy[:], in_=energy[:], func=AF.Tanh)
                # scores [(b%2)*64+s, local pair j]
                psc = pstep.tile([128, 2], F32, tag=f"g1{c}")
                for j in range(2):
                    for kt in range(2):
                        nc.tensor.matmul(
                            psc[:, j:j + 1],
                            lhsT=energy[:, kt, j * 128:(j + 1) * 128],
                            rhs=v_sb[:, kt:kt + 1], start=(kt == 0), stop=(kt == 1))
                exp_f = stp.tile([128, 2], F32, tag=f"exp_f{c}")
                nc.scalar.activation(out=exp_f[:], in_=psc[:], func=AF.Exp)
                pz = pstep.tile([128, 2], F32, tag=f"g1{c}")
                nc.tensor.matmul(pz[:], lhsT=blk[:], rhs=exp_f[:], start=True,
                                 stop=True)
                rcp = stp.tile([128, 2], F32, tag=f"rcp{c}")
                nc.vector.reciprocal(out=rcp[:], in_=pz[:])
                a_eo = a_eo_c[c]
                nc.vector.tensor_tensor(out=a_eo[0:64, :, 0], in0=exp_f[0:64, :],
                                        in1=rcp[0:64, :], op=OP.mult)
                nc.vector.tensor_tensor(out=a_eo[64:128, :, 1], in0=exp_f[64:128, :],
                                        in1=rcp[64:128, :], op=OP.mult)
                # weighted^T [2EH, hb]
                pw = pstep.tile([128, 4, HB], F32, tag=f"g3{c}")
                for j in range(2):
                    b2 = 2 * c + j
                    for et in range(4):
                        nc.tensor.matmul(
                            pw[:, et, 2 * j:2 * j + 2],
                            lhsT=enc_pack[:, b2, et * 128:(et + 1) * 128],
                            rhs=a_eo[:, j, :], start=True, stop=True)
                wdst = xcatT[:, 2:6, t * BL + bb:t * BL + bb + HB]
                nc.vector.tensor_copy(out=wdst, in_=pw[:])
                # gate preacts on PE: rows 0:4 rz-sum, 4:6 i_n, 6:8 gh_n
                pg = pstep.tile([128, 8, HB], F32, tag=f"gA{c}")
                ecol = embT_dec[:, t // 16, (t % 16) * BL + bb:(t % 16) * BL + bb + HB]
                for mt in range(4):
                    for kt in range(2):
                        nc.tensor.matmul(pg[:, mt, :],
                                         lhsT=whhd[:, kt, mt * 128:(mt + 1) * 128],
                                         rhs=h_prev[:, kt, :],
                                         start=(kt == 0), stop=False,
                                         skip_group_check=True)
                    nc.tensor.matmul(pg[:, mt, :],
                                     lhsT=wihe[:, 0, mt * 128:(mt + 1) * 128],
                                     rhs=ecol, start=False, stop=False,
                                     skip_group_check=True)
                    for kt in range(4):
                        nc.tensor.matmul(pg[:, mt, :],
                                         lhsT=wihw[:, kt, mt * 128:(mt + 1) * 128],
                                         rhs=wdst[:, kt, :],
                                         start=False, stop=(kt == 3),
                                         skip_group_check=True)
                for j2, mt in enumerate((4, 5)):   # i_n = Wihe@emb + Wihw@w
                    nc.tensor.matmul(pg[:, 4 + j2, :],
                                     lhsT=wihe[:, 0, mt * 128:(mt + 1) * 128],
                                     rhs=ecol, start=True, stop=False,
                                     skip_group_check=True)
                    for kt in range(4):
                        nc.tensor.matmul(pg[:, 4 + j2, :],
                                         lhsT=wihw[:, kt, mt * 128:(mt + 1) * 128],
                                         rhs=wdst[:, kt, :],
                                         start=False, stop=(kt == 3),
                                         skip_group_check=True)
                for j2, mt in enumerate((4, 5)):   # gh_n
                    for kt in range(2):
                        nc.tensor.matmul(pg[:, 6 + j2, :],
                                         lhsT=whhd[:, kt, mt * 128:(mt + 1) * 128],
                                         rhs=h_prev[:, kt, :],
                                         start=(kt == 0), stop=(kt == 1),
                                         skip_group_check=True)
                # gates (sigmoid via tanh(x/2))
                th = stp.tile([128, 4, HB], F32, tag=f"d_th{c}")
                nc.scalar.activation(out=th[:], in_=pg[:, 0:4, :], func=AF.Tanh,
                                     scale=0.5)
                t_n = stp.tile([128, 2, HB], F32, tag=f"d_n{c}")
                nc.vector.scalar_tensor_tensor(out=t_n[:], in0=th[:, 0:2, :],
                                               scalar=1.0, in1=pg[:, 6:8, :],
                                               op0=OP.add, op1=OP.mult)
                nc.vector.scalar_tensor_tensor(out=t_n[:], in0=t_n[:], scalar=0.5,
                                               in1=pg[:, 4:6, :], op0=OP.mult,
                                               op1=OP.add)
                n_t = stp.tile([128, 2, HB], F32, tag=f"d_tanh{c}")
                nc.scalar.activation(out=n_t[:], in_=t_n[:], func=AF.Tanh)
                d_t = stp.tile([128, 2, HB], F32, tag=f"d_d{c}")
                nc.vector.tensor_tensor(out=d_t[:], in0=h_prev, in1=n_t[:],
                                        op=OP.subtract)
                nc.vector.scalar_tensor_tensor(out=d_t[:], in0=th[:, 2:4, :],
                                               scalar=1.0, in1=d_t[:], op0=OP.add,
                                               op1=OP.mult)
                nc.vector.scalar_tensor_tensor(
                    out=xcatT[:, 0:2, t * BL + bb:t * BL + bb + HB], in0=d_t[:],
                    scalar=0.5, in1=n_t[:], op0=OP.mult, op1=OP.add)

            a_eo_c = []
            for c in range(2):
                ae = sing.tile([128, 2, 2], BF16, tag=f"a_eoc{c}", name=f"a_eoc{c}")
                nc.vector.memset(ae[:], 0.0)
                a_eo_c.append(ae)

            for t in range(TD):
                dec_step(t, 0)
                dec_step(t, 1)
                for g, (tf, t0s, nst) in enumerate(GATHERS):
                    if t == tf:
                        emit_gather(g)
                        fc_queue.extend((g, mt, ns)
                                        for mt in range(nst // 2)
                                        for ns in range(8))
                for _ in range(4):
                    if fc_queue:
                        emit_fc_unit(*fc_queue.pop(0))
            nc.gpsimd.tensor_copy(out=marker[:, 0:1],
                                  in_=xcatT[:, 0, (TD - 1) * BL:(TD - 1) * BL + 1])
            for item in fc_queue:
                emit_fc_unit(*item)

    nc.compile()
    return nc


def _prep_inputs(inputs):
    """Host-side layout prep shared across cores. Returns (shared, per_core)."""
    f = {k: np.asarray(v) for k, v in inputs.items()}
    bf = lambda a: np.ascontiguousarray(a, dtype=np.float32).astype(bfnp)
    tr = lambda a: bf(np.asarray(a, np.float32).T)

    shared = dict(
        enc_emb=bf(f["enc_emb"]),
        dec_emb=bf(f["dec_emb"]),
        wihf_t=tr(f["enc_Wih_f"]), wihb_t=tr(f["enc_Wih_b"]),
        whhf_t=tr(f["enc_Whh_f"]), whhb_t=tr(f["enc_Whh_b"]),
        encfc_t=tr(f["enc_fc_W"]),
        wh_t=tr(f["attn_W"][:, :DH]), we_t=tr(f["attn_W"][:, DH:]),
        v_att=bf(f["attn_v"][0].reshape(2, 128).T),
        wihe_t=tr(f["dec_Wih"][:, :E]), wihw_t=tr(f["dec_Wih"][:, E:]),
        whhd_t=tr(f["dec_Whh"]),
        ident=np.eye(128, dtype=bfnp),
        blkones=np.kron(np.eye(2, dtype=np.float32), np.ones((64, 64), np.float32)),
    )

    src = np.asarray(f["src"])
    trg = np.asarray(f["trg"])
    fcwt_full = tr(f["fc_W"])                                     # [896, 32000]
    per_core = []
    for c in range(NCORES):
        cols = slice(c * BL, (c + 1) * BL)
        si = src[:, cols].astype(np.int32).reshape(-1)            # s-major, 512
        ti = trg[:TD, cols].astype(np.int32).reshape(-1)          # t-major, 504
        ti = np.concatenate([ti, np.zeros(8, np.int32)])
        tok = np.concatenate([si.reshape(4, 128), ti.reshape(4, 128)]).T  # [128, 8]
        per_core.append(dict(
            tok_idx=np.ascontiguousarray(tok),
            fcw_t=np.ascontiguousarray(fcwt_full[:, c * VS:(c + 1) * VS])))
    return shared, per_core


def kernel(**inputs):
    if "nc" not in _CACHE:
        _CACHE["nc"] = _build_program()
    nc = _CACHE["nc"]

    shared, per_core = _prep_inputs(inputs)
    in_maps = [{**shared, **pc} for pc in per_core]

    res = run_bass_kernel_spmd(nc, in_maps, core_ids=list(range(NCORES)))
    _CACHE["last_result"] = res

    out = np.zeros((T, B, V), np.float32)
    for c in range(NCORES):
        out[1:, :, c * VS:(c + 1) * VS] = res.results[c]["out"].reshape(TD, B, VS)
    return out
